# revision 1
# baseline (speedup 1.0000x reference)
"""Trainium2 Bass kernel for nn_GroupLocalSL2 (grouped gather + conv).

out[b,o,i,xo,yo] = sum_{c,f,kh,kw} x[b,c,idx[i,f],xo+kh,yo+kw] * W[o,c,f,kh,kw] + bias[o]

Strategy:
  - Batch B=8 sharded across 8 NeuronCores (data parallel), one b per core.
  - Per core: for each output group i, gather the G_F=7 selected G_IN images
    into SBUF via DMA (idx is read host-side at build time and baked into the
    DMA program). Contraction (c,f)=224 split into chunkA (f=0..3, K=128) and
    chunkB (f=4..6, K=96), partition p = f*32 + c.
  - kw offsets packed into matmul M-blocks: {kw0,kw1} and {kw2,kw3} give
    M=128 matmuls; kw4 runs as two concurrent M=64 col-tiled matmuls. kh
    accumulates in PSUM via row-shifted rhs windows.
  - All 30 matmuls of a row-chunk accumulate into ONE psum tile [128, R, 61]:
    {kw2,kw3} streams x cols 2:63 so it lands at the same psum columns as
    {kw0,kw1}; the kw4 pair lands even-aligned at [0:64] (x cols 4:65 via a
    zero-padded 65th column) and odd-aligned at [64:128] (x cols 3:64).
    Combine is then 2 ops: ScalarE bias-add of the even half [.., 0:60] plus
    VectorE add of the odd half [.., 1:61].
  - Per group, all 8 row-chunks' M=128 matmuls are issued first (phase 1),
    then the M=64 kw4 pairs (phase 2), with one psum bank per row-chunk: the
    PE pays its ~110ns tile-config switch penalty twice per group instead of
    twice per row-chunk.
  - PE warmed up with dummy matmuls during the initial DMA fill (HAM clock
    gate holds the array at 1.2 GHz until ~3.4us of sustained activity);
    group-0 x planes land in two row bands and weights in partition quarters
    so the first real matmul starts as early as possible.
  - Compute in bf16 (host casts x/W), fp32 PSUM accumulate; rel err ~2e-3.
  - HW exec: ~618us (baseline 638us): ~594us TensorE streaming (96% busy,
    zero gaps, at the serial column-stream floor of this decomposition:
    25.4 cols/output px vs the 21.9 bf16 MAC bound, the gap being the
    unavoidable K=224/256 underfill), ~16us HBM-bound input fill (overlapped
    with warmup matmuls), ~8us fixed end-of-program semaphore teardown.

  Falsified alternatives (measured on HW, all reverted — do not retry):
  - Per-kw-block weight tiles (wa0/wa1/..): +120us — splitting the combined
    weight tiles breaks LDWEIGHTS/FWL overlap (+42ns on EVERY matmul).
  - Host-prewindowed pitch-61 x variants for flat (crossing-free) rhs:
    +152us — the 2.8x gather traffic stalls matmuls; prefetch cannot hide it.
  - Per-kh weight DMA slices / 8-way splits / separate top-bottom group-0 x
    tiles: +2..7us — dma_start dispatch is ~620ns serial on SyncE, and small
    strided transfers run far below the ~350GB/s aggregate of large ones.
  - fp8 (any hi/lo split passing rel-err 2e-2 is >=1.5x bf16 MACs), Winograd
    (transforms cannot ride the PE; DVE is 100x too slow), kh-in-K packing
    (needs 5x row-shifted x copies): all slower at equal accuracy.
"""

import os
import sys

import numpy as np
import ml_dtypes

for _p in ("/opt/trn_rl_repo", "/root/.axon_site/_ro/trn_rl_repo"):
    if os.path.isdir(_p) and _p not in sys.path:
        sys.path.append(_p)

import concourse.bass as bass
import concourse.mybir as mybir
import concourse.tile as tile
from concourse import bacc
from concourse.bass_utils import run_bass_kernel_spmd

BF16 = ml_dtypes.bfloat16

B, C, G_IN = 8, 32, 33
O, G_F, KH, KW = 64, 7, 5, 5
X, Y = 64, 64
G_OUT = 15
XO, YO = X - KH + 1, Y - KW + 1  # 60, 60
RCH = 8  # output rows per chunk (8*61 = 488 <= 512 psum bank)
N_WARM = 60  # dummy matmuls to flip the HAM clock gate during DMA fill


def _build_nc(idx, n_groups=G_OUT):
    """Build the single-core Bass program (idx values baked into DMAs)."""
    nc = bacc.Bacc("TRN2", target_bir_lowering=False, debug=False)
    dt = mybir.dt
    xin = nc.dram_tensor("x", [C, G_IN, X, Y + 1], dt.bfloat16, kind="ExternalInput")
    wa_d = nc.dram_tensor("wa", [128, KH, 5 * O], dt.bfloat16, kind="ExternalInput")
    wb_d = nc.dram_tensor("wb", [96, KH, 5 * O], dt.bfloat16, kind="ExternalInput")
    bias_d = nc.dram_tensor("bias", [O, 1], dt.float32, kind="ExternalInput")
    out_d = nc.dram_tensor("out", [O, G_OUT, XO, YO], dt.float32, kind="ExternalOutput")

    rchunks = [(r0, min(RCH, XO - r0)) for r0 in range(0, XO, RCH)]

    with tile.TileContext(nc) as tc:
        with (
            tc.tile_pool(name="wpool", bufs=1) as wpool,
            tc.tile_pool(name="warm", bufs=1) as warmpool,
            tc.tile_pool(name="xpool", bufs=2) as xpool,
            tc.tile_pool(name="tpool", bufs=3) as tpool,
            tc.tile_pool(name="opool", bufs=4) as opool,
            tc.tile_pool(name="psum", bufs=7, space="PSUM") as pp,
            tc.tile_pool(name="psumt", bufs=1, space="PSUM") as ppt,
        ):
            # PE warmup: the HAM clock gate holds the PE at 1.2 GHz until it
            # sees ~3.4us of sustained activity. Burn that in on garbage data
            # while the weight/x DMAs fill SBUF.
            wmt = warmpool.tile([128, 256], dt.bfloat16, tag="warm")
            nc.vector.memset(wmt[:, :], 0.0)

            # wa is split into the kw-pair blocks (needed by the very
            # first matmuls) and the kw4 block (not needed until phase 2),
            # and further into partition quarters, so the critical weight
            # bytes land as early as possible across DMA queues.
            wa01 = wpool.tile([128, KH, 4 * O], dt.bfloat16, tag="wa01")
            wa4 = wpool.tile([128, KH, O], dt.bfloat16, tag="wa4")
            wb = wpool.tile([96, KH, 5 * O], dt.bfloat16, tag="wb")
            bias_sb = wpool.tile([O, 1], dt.float32, tag="bias")
            for q in range(4):
                nc.sync.dma_start(
                    wa01[q * 32 : (q + 1) * 32, :, :],
                    wa_d[q * 32 : (q + 1) * 32, :, 0 : 4 * O],
                )

            for i in range(n_groups):
                # xa has a 65th zeroed column so the kw4 matmul can stream a
                # full 61-wide window (cols 4:65) for a contiguous psum write.
                xa = xpool.tile([128, X, Y + 1], dt.bfloat16, tag="xa")
                xb = xpool.tile([96, X, Y + 1], dt.bfloat16, tag="xb")
                # group 0 in two row bands: smaller transfers finish sooner
                # after their serial ~620ns dma_start dispatch slots
                bands = ((0, 24), (24, X)) if i == 0 else ((0, X),)
                for bi, (lo, hi) in enumerate(bands):
                    for f in range(G_F):
                        g = int(idx[i, f])
                        if f < 4:
                            nc.sync.dma_start(
                                xa[f * 32 : (f + 1) * 32, lo:hi, :],
                                xin[:, g, lo:hi, :],
                            )
                        else:
                            fb = f - 4
                            nc.sync.dma_start(
                                xb[fb * 32 : (fb + 1) * 32, lo:hi, :],
                                xin[:, g, lo:hi, :],
                            )
                    if i == 0 and bi == 0:
                        nc.sync.dma_start(wb[0:48, :, :], wb_d[0:48, :, :])
                        nc.sync.dma_start(wb[48:96, :, :], wb_d[48:96, :, :])
                        nc.sync.dma_start(wa4[:, :, :], wa_d[:, :, 4 * O : 5 * O])
                        nc.sync.dma_start(bias_sb[:, :], bias_d[:, :])
                if i == 0:
                    # warmup matmuls run while the DMAs above land
                    wps = ppt.tile([128, 4, 61], dt.float32, tag="pt")
                    for _ in range(N_WARM):
                        nc.tensor.matmul(
                            wps[:, :, :],
                            wmt[:, 0:128],
                            wmt[:, 0:244],
                            start=True,
                            stop=True,
                        )

                # Phase 1: the M=128 {kw0,kw1}/{kw2,kw3} blocks of ALL row
                # chunks, one psum bank per chunk. Phase 2: the M=64 kw4
                # col-tiled pairs. Keeping all M=128 matmuls together avoids
                # the ~110ns PE tile-config switch penalty on every M change
                # (2 per group instead of 2 per row chunk).
                ptiles = []
                for r0, R in rchunks:
                    tail = R != RCH
                    p = (ppt if tail else pp).tile(
                        [128, R, 61], dt.float32, tag="pt" if tail else "p"
                    )
                    ptiles.append(p)
                    # {kw2,kw3} streams x cols 2:63 so its contributions land
                    # at the same psum columns as {kw0,kw1}.
                    for grp in (0, 1):
                        c0 = 2 * grp
                        for ci, (xt, wt, Kc) in enumerate(
                            ((xa, wa01, 128), (xb, wb, 96))
                        ):
                            for kh in range(KH):
                                nc.tensor.matmul(
                                    p[:, 0:R, :],
                                    wt[0:Kc, kh, grp * 128 : grp * 128 + 128],
                                    xt[0:Kc, r0 + kh : r0 + kh + R, c0 : c0 + 61],
                                    start=(grp == 0 and ci == 0 and kh == 0),
                                    stop=False,
                                )
                for (r0, R), p in zip(rchunks, ptiles):
                    # kw4 col-tiled pair: even-aligned at [0:64] (x cols 4:65,
                    # zero-padded 65th col), odd-aligned at [64:128] (cols
                    # 3:64).
                    for kh in range(KH):
                        nc.tensor.matmul(
                            p[0:64, 0:R, :],
                            wa4[0:128, kh, 0:64],
                            xa[0:128, r0 + kh : r0 + kh + R, 4:65],
                            start=False,
                            stop=False,
                        )
                        nc.tensor.matmul(
                            p[64:128, 0:R, :],
                            wb[0:96, kh, 256:320],
                            xb[0:96, r0 + kh : r0 + kh + R, 3:64],
                            start=False,
                            stop=(kh == KH - 1),
                        )

                    # Combine (one PSUM operand per instruction): bias-add of
                    # the even half on ScalarE, odd half on VectorE.
                    t = tpool.tile([O, RCH, 60], dt.float32, tag="t")
                    ot = opool.tile([O, RCH, 60], dt.float32, tag="out")
                    nc.scalar.add(t[:, 0:R, :], p[0:64, 0:R, 0:60], bias_sb[:, 0:1])
                    nc.vector.tensor_add(
                        ot[:, 0:R, :], t[:, 0:R, :], p[64:128, 0:R, 1:61]
                    )
                    if i == n_groups - 1:
                        # last group: split transfers so the drain after the
                        # final matmuls is short
                        rh = max(R // 2, 1)
                        nc.sync.dma_start(
                            out_d[:, i, r0 : r0 + rh, :], ot[:, 0:rh, :]
                        )
                        nc.sync.dma_start(
                            out_d[:, i, r0 + rh : r0 + R, :], ot[:, rh:R, :]
                        )
                    else:
                        nc.sync.dma_start(
                            out_d[:, i, r0 : r0 + R, :], ot[:, 0:R, :]
                        )
    nc.compile()
    return nc


def _prep_inputs(x, weight, bias, idx):
    """Host-side staging: bf16 cast + lhsT weight layout, per-core in_maps."""
    x16 = np.asarray(x).astype(BF16)  # [B, C, G_IN, X, Y]
    # pad a zero 65th column so kw4's 61-wide window (cols 4:65) exists
    x16 = np.pad(x16, ((0, 0), (0, 0), (0, 0), (0, 0), (0, 1)))
    w = np.asarray(weight).astype(np.float32)
    # lhsT layout: partition p = f*32 + c (within chunk), free = [kh, kw*64+o]
    wt = w.transpose(2, 1, 3, 4, 0)  # [G_F, C, KH, KW, O]
    wa = np.ascontiguousarray(wt[0:4].reshape(128, KH, KW * O)).astype(BF16)
    wb = np.ascontiguousarray(wt[4:7].reshape(96, KH, KW * O)).astype(BF16)
    b2 = np.ascontiguousarray(np.asarray(bias).astype(np.float32).reshape(O, 1))
    in_maps = []
    for b in range(B):
        in_maps.append(
            {
                "x": np.ascontiguousarray(x16[b]),
                "wa": wa,
                "wb": wb,
                "bias": b2,
            }
        )
    return in_maps


def run(x, weight, bias, idx, trace=False):
    idx = np.asarray(idx).astype(np.int64)
    assert idx.shape == (G_OUT, G_F) and idx.min() >= 0 and idx.max() < G_IN
    nc = _build_nc(idx)
    in_maps = _prep_inputs(x, weight, bias, idx)
    res = run_bass_kernel_spmd(nc, in_maps, list(range(B)), trace=trace)
    out = np.stack([res.results[b]["out"] for b in range(B)]).astype(np.float32)
    return out, res


def kernel(x, weight, bias, idx):
    out, _ = run(x, weight, bias, idx, trace=False)
    return out

